# revision 10
# baseline (speedup 1.0000x reference)
"""Trainium2 Bass kernel for the MERITS_T patient model (B=1024 data-parallel
over 8 cores), collective-free.

Mathematical simplification of the reference (verified to ~3e-7 rel err fp32):
  - E_de softmaxes over a single key -> GATs / graph-MHA / drug_mem are dead
    code; e0 needs only attention query row 0, i.e. only med[:, 0, :].
  - The static half of patient_j is visit-independent -> softmax-invariant in
    the logits; its attention-weighted value is `static` and re-enters linearly
    via SS = sum_h MW_h[32:64].
  - The gate sigma(x.glu_gate) multiplies logits and values linearly; folded in
    as scalars around the softmax.
  - relu(final) @ out_w1 = relu(r) @ (sum_m out_w1[m]): the 43MB out_w1 only
    enters via its m-block sum W1sum [64, 1160].

Distribution: on this platform ANY firmware collective costs ~90us wall
(model-entry barrier + RDH protocol + launch skew; measured on a trivial
AllGather), and the remote-DMA ISA extension does not compile.  So the kernel
is fully data-parallel with zero cross-core traffic: every core reads the FULL
out_w1 in fp8 (x256, one e4m3 rounding; adds ~4.5e-3 output rel err vs the
2e-2 gate) and reduces the 145 m-blocks itself on the PE via identity-matmul
accumulation into PSUM (m-parity pairs folded into the 128 partitions; the
final MLP contracts both parities at once by duplicating relu(r) rows).
Everything else runs in bf16; sigmoids run as tanh on ScalarE (avoids an
activation-table swap before the Exp).
"""

import numpy as np
import ml_dtypes

import concourse.bass as bass
import concourse.mybir as mybir
from concourse.bass_utils import run_bass_kernel_spmd
from concourse.tile import TileContext

F32 = mybir.dt.float32
BF16 = mybir.dt.bfloat16
FP8 = mybir.dt.float8e4
AF = mybir.ActivationFunctionType
ALU = mybir.AluOpType
AX = mybir.AxisListType


def split_multi_waits(nc):
    """The walrus on this image encodes at most ONE sync wait per TPB
    instruction. Hoist excess waits onto standalone InstEventSemaphore ops."""
    wid = 0
    for f in nc.m.functions:
        for bb in f.blocks:
            out = []
            for ins in bb.instructions:
                si = ins.sync_info
                if si is not None and si.on_wait and len(si.on_wait) > 1:
                    waits = list(si.on_wait)
                    for w in waits[:-1]:
                        wid += 1
                        out.append(mybir.InstEventSemaphore(
                            name=f"Wsplit-{wid}", engine=ins.engine,
                            ins=[], outs=[],
                            sync_info=mybir.SyncInfo(on_wait=[w], on_update=[])))
                    si.on_wait = waits[-1:]
                out.append(ins)
            bb.instructions = out
    return wid


B, T, MED, LAB, GLU, D, H = 1024, 25, 145, 1956, 16, 64, 32
NH, DH = 4, 16
NC_CORES = 8
BC = B // NC_CORES       # 128 patients per core
HID = MED * D // 8       # 1160
KLAB = 16                # 2048 = 16*128 lab contraction tiles
MP = 73                  # m-pairs: 146 m-slots = 145 real + 1 zero pad
TP = T + 1               # 26, padded visit dim for the j-reduce
W1SCALE = 256.0          # fp8 pre-scale (out_w1 sigma~0.01 is subnormal in e4m3)

# column offsets inside the packed small-weight slab [128, PCOLS] (bf16)
_PC = {}
_o = 0
for _name, _w in [("ident", 128), ("woT", D), ("m2wvT", D), ("m2wo", D),
                  ("mwsb", D), ("mw2sb", D), ("mgT", 1), ("w2sb", H),
                  ("gw3", 2 * H), ("wqT4", NH * D), ("wkT4", NH * D),
                  ("wvT4", NH * D), ("wbd", 16 * H)]:
    _PC[_name] = (_o, _o + _w)
    _o += _w
PCOLS = _o


def build_bass():
    nc = bass.Bass()

    def inp(name, shape, dt=F32):
        return nc.dram_tensor(name, list(shape), dt, kind="ExternalInput")

    slab_d = inp("slab", (128, MP * 1000), FP8)     # out_w1 cols 0:1000, fp8 x256
    slabV_d = inp("slabV", (128, 160 * MP), FP8)   # cols 1000:1160, mp innermost
    id8_d = inp("id8", (128, 128), FP8)            # fp8 identity
    labT_d = inp("labT", (128, KLAB * BC), BF16)    # lab^T partition-major
    gluT_d = inp("gluT", (128, 4 * BC), BF16)       # glu partition-major
    tfT_d = inp("tfT", (128, 4 * BC), BF16)         # time_feat, same layout
    med0T_d = inp("med0T", (MED + 1, BC))           # med visit-0 ^T + ones row
    packB_d = inp("packB", (128, PCOLS), BF16)      # small weights, packed
    rows_d = inp("rowsB", (1, 8 * H + H), BF16)     # glu_b x8 | glu_gate
    outb1_d = inp("outb1T", (128, 10))              # out_b1 as [p, t]
    ow2sb_d = inp("ow2sbH", (128, 10 * MED), BF16)  # out_w2+b2 as [k, (t n)]
    w1sb_d = inp("w1sbH", (128, KLAB * D), BF16)    # sll_w1+b1 as [k, (t d)]
    out_d = nc.dram_tensor("out", [BC, MED], F32, kind="ExternalOutput")

    with TileContext(nc) as tc, \
            tc.tile_pool(name="consts", bufs=1) as cp, \
            tc.tile_pool(name="pa", bufs=5, space="PSUM") as pa, \
            tc.tile_pool(name="ps1", bufs=1, space="PSUM") as ps1, \
            tc.tile_pool(name="pw", bufs=1, space="PSUM") as pw:

        dmaA = nc.scalar.dma_start   # qAct ring: everything but the slab
        dmaS = nc.sync.dma_start     # qSP ring: the big fp8 slab + output

        # ================= input DMAs ====================================
        # all DRAM layouts are partition-major (host-marshalled), so every
        # transfer is a plain 2D copy with a short descriptor list.
        identF8 = cp.tile([128, 128], FP8, tag="identF8")
        dmaA(out=identF8, in_=id8_d[:])
        slab = cp.tile([128, MP, 1000], FP8, tag="slab")
        slab_v = slab_d[:].rearrange("p (m j) -> p m j", j=1000)
        slabV = cp.tile([128, 160, MP], FP8, tag="slabV")
        bnds = [0, 2, 5, 9, 14, 19, 24, 30, 36, 42, 48, 54, 60, 66, 73]
        NCH = len(bnds) - 1

        def slab_dma(q):
            lo, hi = bnds[q], bnds[q + 1]
            dmaS(out=slab[:, lo:hi, :], in_=slab_v[:, lo:hi, :])

        for q in range(NCH):
            slab_dma(q)
        gluT = cp.tile([128, 4, BC], BF16, tag="gluT")
        dmaA(out=gluT, in_=gluT_d[:].rearrange("k (c p) -> k c p", p=BC))
        tfT = cp.tile([128, 4, BC], BF16, tag="tfT")
        dmaA(out=tfT, in_=tfT_d[:].rearrange("k (c p) -> k c p", p=BC))
        rows = cp.tile([1, 8 * H + H], BF16, tag="rows")
        dmaA(out=rows, in_=rows_d[:])
        ggb = cp.tile([128, H], BF16, tag="ggb")
        dmaA(out=ggb, in_=rows_d[0:1, 8 * H:8 * H + H].broadcast_to((128, H)))
        pack = cp.tile([128, PCOLS], BF16, tag="pack")
        dmaA(out=pack, in_=packB_d[:])
        med0Ta = cp.tile([128, BC], F32, tag="med0Ta")
        dmaA(out=med0Ta, in_=med0T_d[0:128, :])
        med0Tb = cp.tile([18, BC], F32, tag="med0Tb")
        dmaA(out=med0Tb, in_=med0T_d[128:MED + 1, :])
        w1sb = cp.tile([128, KLAB, D], BF16, tag="w1sb")
        dmaA(out=w1sb, in_=w1sb_d[:].rearrange("k (t d) -> k t d", d=D))
        labT = cp.tile([128, KLAB, BC], BF16, tag="labT")
        dmaA(out=labT, in_=labT_d[:].rearrange("k (t p) -> k t p", p=BC))
        outb1T = cp.tile([128, 10], F32, tag="outb1T")
        dmaA(out=outb1T, in_=outb1_d[:])
        slabV_v = slabV_d[:].rearrange("k (j m) -> k j m", m=MP)
        dmaA(out=slabV[:, 0:80, :], in_=slabV_v[:, 0:80, :])
        dmaA(out=slabV[:, 80:160, :], in_=slabV_v[:, 80:160, :])
        ow2sb = cp.tile([128, 10, MED], BF16, tag="ow2sb")
        dmaA(out=ow2sb, in_=ow2sb_d[:].rearrange("k (t n) -> k t n", n=MED))

        def pk(name, nrows):
            lo, hi = _PC[name]
            return pack[0:nrows, lo:hi]

        identb = pk("ident", 128)
        wbd = pk("wbd", 128).rearrange("k (t h) -> k t h", h=H)
        gw3 = pk("gw3", GLU)
        gb8 = rows[0:1, 0:8 * H]

        ones1b = cp.tile([1, 128], BF16, tag="ones1b")
        nc.vector.memset(ones1b, 1.0)

        # ================= W1sum accumulate helper =======================
        acc = pw.tile([128, 1000], F32, tag="acc")
        JCH = [(0, 512), (512, 1000)]

        def slab_block(q):
            for mp in range(bnds[q], bnds[q + 1]):
                for (jl, jh) in JCH:
                    nc.tensor.matmul(acc[:, jl:jh], lhsT=identF8,
                                     rhs=slab[:, mp, jl:jh],
                                     start=(mp == 0), stop=(mp == MP - 1))

        slab_block(0)

        # ================= glu encoder x = tanh(glu_in @ glu_w + b) ======
        x_sbb = cp.tile([128, T, H], BF16, tag="x_sbb")
        xTb = cp.tile([128, H, TP], BF16, tag="xTb")
        nc.vector.memset(xTb[:, :, T:TP], 0.0)
        for c in range(3):
            gx = pa.tile([128, 8, H], F32, tag="pp")
            nc.tensor.matmul(gx, lhsT=gluT[:, c, :], rhs=wbd[:, 0:8, :],
                             start=True, stop=False)
            nc.tensor.matmul(gx, lhsT=tfT[:, c, :], rhs=wbd[:, 8:16, :],
                             start=False, stop=False)
            nc.tensor.matmul(gx, lhsT=ones1b[0:1, :],
                             rhs=gb8.rearrange("a (t h) -> a t h", h=H),
                             start=False, stop=True)
            nc.scalar.activation(out=x_sbb[:, 8 * c:8 * c + 8, :], in_=gx,
                                 func=AF.Tanh)
            nc.scalar.activation(out=xTb[:, :, 8 * c:8 * c + 8],
                                 in_=gx.rearrange("p j f -> p f j"),
                                 func=AF.Tanh)
        gx3 = pa.tile([128, 1, H], F32, tag="pp")
        nc.tensor.matmul(gx3[:, 0, :], lhsT=gluT[0:GLU, 3, :], rhs=gw3[:, 0:H],
                         start=True, stop=False)
        nc.tensor.matmul(gx3[:, 0, :], lhsT=tfT[0:GLU, 3, :], rhs=gw3[:, H:2 * H],
                         start=False, stop=False)
        nc.tensor.matmul(gx3[:, 0, :], lhsT=ones1b[0:1, :], rhs=gb8[0:1, 0:H],
                         start=False, stop=True)
        nc.scalar.activation(out=x_sbb[:, 24:25, :], in_=gx3, func=AF.Tanh)
        nc.scalar.activation(out=xTb[:, :, 24:25],
                             in_=gx3.rearrange("p a f -> p f a"), func=AF.Tanh)

        slab_block(1)

        # ================= weight prep on PE (bf16) ======================
        wvo_ps = pa.tile([D, D], F32, tag="pp")
        nc.tensor.matmul(wvo_ps, lhsT=pk("m2wvT", D), rhs=pk("m2wo", D))
        wvo2b = cp.tile([D, D], BF16, tag="wvo2b")
        nc.scalar.copy(out=wvo2b, in_=wvo_ps)
        woT = pk("woT", D)
        wov_ps = pa.tile([DH, NH, D], F32, tag="pp")
        for h in range(NH):
            nc.tensor.matmul(wov_ps[:, h, :], lhsT=woT[:, h * DH:(h + 1) * DH],
                             rhs=wvo2b[:])
        wov4 = cp.tile([DH, NH, D], BF16, tag="wov4")
        nc.scalar.copy(out=wov4, in_=wov_ps)
        wvT4 = pk("wvT4", DH).rearrange("c (h d) -> c h d", h=NH)
        mw_ps = pa.tile([H, NH, D], F32, tag="pp")
        for h in range(NH):
            nc.tensor.matmul(mw_ps[:, h, :], lhsT=wvT4[:, h, 0:H],
                             rhs=wov4[:, h, :])
        mw4b = cp.tile([H, NH, D], BF16, tag="mw4b")
        nc.scalar.copy(out=mw4b, in_=mw_ps)
        ss_ps = pa.tile([H, D], F32, tag="pp")
        for h in range(NH):
            nc.tensor.matmul(ss_ps, lhsT=wvT4[:, h, H:D], rhs=wov4[:, h, :],
                             start=(h == 0), stop=(h == NH - 1))
        ss_b = cp.tile([H, D], BF16, tag="ss_b")
        nc.scalar.copy(out=ss_b, in_=ss_ps)
        wqT4 = pk("wqT4", DH).rearrange("c (h d) -> c h d", h=NH)
        wkT4 = pk("wkT4", DH).rearrange("c (h d) -> c h d", h=NH)
        ahg_ps = pa.tile([D, NH, H], F32, tag="pp")
        for h in range(NH):
            nc.tensor.matmul(ahg_ps[:, h, :], lhsT=wqT4[:, h, :],
                             rhs=wkT4[:, h, 0:H])
        ahgb = cp.tile([D, NH, H], BF16, tag="ahgb")
        nc.scalar.activation(out=ahgb, in_=ahg_ps, func=AF.Copy,
                             scale=1.0 / DH ** 0.5)

        slab_block(2)

        # ================= med visit-0 encoder (transposed) ==============
        mbTa = cp.tile([128, BC], BF16, tag="mbTa")
        nc.vector.tensor_scalar(out=mbTa, in0=med0Ta, scalar1=0.9, scalar2=None,
                                op0=ALU.is_gt)
        mbTb = cp.tile([18, BC], BF16, tag="mbTb")
        nc.vector.tensor_scalar(out=mbTb, in0=med0Tb, scalar1=0.9, scalar2=None,
                                op0=ALU.is_gt)
        x0_ps = pa.tile([D, BC], F32, tag="pp")
        nc.tensor.matmul(x0_ps, lhsT=pk("mwsb", 128), rhs=mbTa[:],
                         start=True, stop=False)
        nc.tensor.matmul(x0_ps, lhsT=pk("mw2sb", 18), rhs=mbTb[:],
                         start=False, stop=True)
        x0b = cp.tile([D, BC], BF16, tag="x0b")
        nc.vector.tensor_copy(out=x0b, in_=x0_ps)
        g0_ps = pa.tile([1, BC], F32, tag="pp")
        nc.tensor.matmul(g0_ps, lhsT=pk("mgT", D), rhs=x0b[:])
        # sigmoid(z) = 0.5*tanh(z/2) + 0.5 (keeps ScalarE on the tanh table)
        tg0 = cp.tile([1, BC], F32, tag="tg0")
        nc.scalar.activation(out=tg0, in_=g0_ps, func=AF.Tanh, scale=0.5)
        sg0b = cp.tile([1, BC], BF16, tag="sg0b")
        nc.vector.tensor_scalar(out=sg0b, in0=tg0, scalar1=0.5, scalar2=0.5,
                                op0=ALU.mult, op1=ALU.add)
        sg0r_ps = pa.tile([D, BC], F32, tag="pp")
        nc.tensor.matmul(sg0r_ps, lhsT=ones1b[0:1, 0:D], rhs=sg0b[:])
        mr0b = cp.tile([D, BC], BF16, tag="mr0b")
        nc.vector.tensor_mul(mr0b, x0b, sg0r_ps)
        u_ps = pa.tile([BC, NH, H], F32, tag="pp")
        nc.tensor.matmul(u_ps, lhsT=mr0b[:],
                         rhs=ahgb[:].rearrange("d h f -> d (h f)"))
        u_bb = cp.tile([BC, NH, H], BF16, tag="u_bb")
        nc.vector.tensor_copy(out=u_bb, in_=u_ps)

        # ================= gate = sigmoid(x . glu_gate) ==================
        gm = cp.tile([128, T, H], BF16, tag="gm")
        nc.vector.tensor_mul(gm, x_sbb,
                             ggb[:].unsqueeze(1).broadcast_to((128, T, H)))
        gs = cp.tile([128, T], F32, tag="gs")
        nc.vector.tensor_reduce(out=gs, in_=gm, axis=AX.X, op=ALU.add)
        gth = cp.tile([128, T], F32, tag="gth")
        nc.scalar.activation(out=gth, in_=gs, func=AF.Tanh, scale=0.5)
        gate = cp.tile([128, T], F32, tag="gate")
        nc.vector.tensor_scalar(out=gate, in0=gth, scalar1=0.5, scalar2=0.5,
                                op0=ALU.mult, op1=ALU.add)

        # ================= one-query attention (glu half only) ===========
        sprod = cp.tile([128, NH, T, H], BF16, tag="sprod")
        nc.vector.tensor_mul(
            sprod,
            x_sbb[:].unsqueeze(1).broadcast_to((128, NH, T, H)),
            u_bb[:].unsqueeze(2).broadcast_to((128, NH, T, H)))
        s4 = cp.tile([128, NH, T], F32, tag="s4")
        nc.vector.tensor_reduce(out=s4.rearrange("p h j -> p (h j)"),
                                in_=sprod.rearrange("p h j f -> p (h j) f"),
                                axis=AX.X, op=ALU.add)
        sg4 = cp.tile([128, NH, T], F32, tag="sg4")
        nc.vector.tensor_mul(sg4, s4,
                             gate[:].unsqueeze(1).broadcast_to((128, NH, T)))
        es = cp.tile([128, NH, TP], BF16, tag="es")
        nc.vector.memset(es[:, :, T:TP], 0.0)
        nc.scalar.activation(out=es[:, :, 0:T], in_=sg4, func=AF.Exp)
        den = cp.tile([128, NH], F32, tag="den")
        nc.vector.tensor_reduce(out=den, in_=es[:, :, 0:T], axis=AX.X,
                                op=ALU.add)
        rden = cp.tile([128, NH], F32, tag="rden")
        nc.vector.reciprocal(out=rden, in_=den)
        cgb = cp.tile([128, NH, TP], BF16, tag="cgb")
        nc.vector.tensor_mul(cgb[:, :, 0:T], es[:, :, 0:T],
                             gate[:].unsqueeze(1).broadcast_to((128, NH, T)))
        coefb = cp.tile([128, NH, TP], BF16, tag="coefb")
        nc.vector.memset(coefb[:, :, T:TP], 0.0)
        nc.vector.tensor_mul(coefb[:, :, 0:T], cgb[:, :, 0:T],
                             rden[:].unsqueeze(2).broadcast_to((128, NH, T)))
        wprod = cp.tile([128, NH, H, TP], BF16, tag="wprod")
        nc.vector.tensor_mul(
            wprod,
            coefb[:].unsqueeze(2).broadcast_to((128, NH, H, TP)),
            xTb[:].unsqueeze(1).broadcast_to((128, NH, H, TP)))
        y4 = cp.tile([128, NH, H], F32, tag="y4")
        nc.vector.tensor_reduce(out=y4.rearrange("p h f -> p (h f)"),
                                in_=wprod.rearrange("p h f j -> p (h f) j"),
                                axis=AX.X, op=ALU.add)
        y4b = cp.tile([128, NH, H], BF16, tag="y4b")
        nc.vector.tensor_copy(out=y4b, in_=y4)
        accV = cp.tile([128, 160], F32, tag="accV")
        nc.vector.tensor_reduce(out=accV[:, 0:80], in_=slabV[:, 0:80, :],
                                axis=AX.X, op=ALU.add)
        nc.vector.tensor_reduce(out=accV[:, 80:160], in_=slabV[:, 80:160, :],
                                axis=AX.X, op=ALU.add)

        # ================= static MLP over lab ===========================
        st1_ps = ps1.tile([D, BC], F32, tag="st1")
        for t in range(KLAB):
            nc.tensor.matmul(st1_ps, lhsT=w1sb[:, t, :], rhs=labT[:, t, :],
                             start=(t == 0), stop=(t == KLAB - 1))
        st1rb = cp.tile([D + 1, BC], BF16, tag="st1rb")
        nc.scalar.activation(out=st1rb[0:D, :], in_=st1_ps, func=AF.Relu)
        nc.vector.memset(st1rb[D:D + 1, :], 1.0)
        st2_ps = pa.tile([H, BC], F32, tag="pp")
        nc.tensor.matmul(st2_ps, lhsT=pk("w2sb", D + 1), rhs=st1rb[:])
        staticb = cp.tile([H, BC], BF16, tag="staticb")
        nc.scalar.activation(out=staticb, in_=st2_ps, func=AF.Relu)

        for q in range(3, 10):
            slab_block(q)

        # attention tail interleaved while the last slab chunks stream in
        yt_ps = pa.tile([H, NH, BC], BF16, tag="pp")
        for h in range(NH):
            nc.tensor.transpose(yt_ps[:, h, :], y4b[:, h, :], identb[:])
        yT4b = cp.tile([H, NH, BC], BF16, tag="yT4b")
        nc.scalar.copy(out=yT4b, in_=yt_ps)
        rT_ps = pa.tile([D, BC], F32, tag="pp")
        for h in range(NH):
            nc.tensor.matmul(rT_ps, lhsT=mw4b[:, h, :], rhs=yT4b[:, h, :],
                             start=(h == 0), stop=False)
        nc.tensor.matmul(rT_ps, lhsT=ss_b[:], rhs=staticb[:],
                         start=False, stop=True)
        # relu(r)^T duplicated into both parity halves for the mm1 contraction
        rrT2 = cp.tile([128, BC], BF16, tag="rrT2")
        nc.scalar.activation(out=rrT2[0:D, :], in_=rT_ps, func=AF.Relu)
        nc.scalar.activation(out=rrT2[D:128, :], in_=rT_ps, func=AF.Relu)

        for q in range(10, NCH):
            slab_block(q)

        # ================= final MLP =====================================
        w1bb = cp.tile([128, HID], BF16, tag="w1bb")
        nc.vector.tensor_copy(out=w1bb[:, 1000:HID], in_=accV)
        nc.scalar.copy(out=w1bb[:, 0:512], in_=acc[:, 0:512])
        nc.scalar.copy(out=w1bb[:, 512:1000], in_=acc[:, 512:1000])
        hidT = cp.tile([128, 10, BC], BF16, tag="hidT")
        nc.vector.memset(hidT[:, 9, :], 1.0)
        for t in range(10):
            n = 128 if t < 9 else 8
            h_ps = pa.tile([128, BC], F32, tag="pp")
            nc.tensor.matmul(h_ps[0:n, :], lhsT=w1bb[:, t * 128:t * 128 + n],
                             rhs=rrT2[:])
            if t % 2 == 0:
                nc.scalar.activation(out=hidT[0:n, t, :], in_=h_ps[0:n, :],
                                     func=AF.Relu, scale=1.0 / W1SCALE,
                                     bias=outb1T[0:n, t:t + 1])
            else:
                nc.vector.scalar_tensor_tensor(
                    out=hidT[0:n, t, :], in0=h_ps[0:n, :],
                    scalar=1.0 / W1SCALE,
                    in1=outb1T[0:n, t:t + 1].broadcast_to((n, BC)),
                    op0=ALU.mult, op1=ALU.add)
                nc.vector.tensor_scalar(out=hidT[0:n, t, :],
                                        in0=hidT[0:n, t, :], scalar1=0.0,
                                        scalar2=None, op0=ALU.max)
        out_ps = ps1.tile([BC, MED], F32, tag="st1")
        for t in range(10):
            k = 128 if t < 9 else 9
            nc.tensor.matmul(out_ps, lhsT=hidT[0:k, t, :], rhs=ow2sb[0:k, t, :],
                             start=(t == 0), stop=(t == 9))
        out_sb = cp.tile([BC, MED], F32, tag="out_sb")
        nc.vector.tensor_copy(out=out_sb, in_=out_ps)
        dmaS(out=out_d[:], in_=out_sb)

    split_multi_waits(nc)
    return nc


_CACHED_NC = None


def make_in_maps(inputs):
    """Pure input marshalling: transpose / reshape / concat / pad / cast only."""
    f = lambda x: np.ascontiguousarray(np.asarray(x, dtype=np.float32))
    cat = np.concatenate
    bf = lambda x: np.ascontiguousarray(np.asarray(x).astype(ml_dtypes.bfloat16))

    lab = f(inputs["lab"])
    glu = f(inputs["glu"]).reshape(B, T * GLU)
    tf = f(inputs["time_feat"]).reshape(B, T * GLU)
    med0 = f(inputs["med"])[:, 0, :]

    # full out_w1 -> fp8 x256, m-parity folded into partitions: [128, mp, j]
    w1b = f(inputs["out_w1"]).reshape(MED, D, HID)
    w1p = np.zeros((2 * MP, D, HID), np.float32)
    w1p[0:MED] = w1b
    arr = (w1p.reshape(MP, 2, D, HID).transpose(1, 2, 0, 3)
           .reshape(128, MP, HID) * W1SCALE)
    slab = np.ascontiguousarray(
        arr[:, :, 0:1000].reshape(128, MP * 1000)
        .astype(ml_dtypes.float8_e4m3))
    slabV = np.ascontiguousarray(
        arr[:, :, 1000:HID].transpose(0, 2, 1).reshape(128, 160 * MP)
        .astype(ml_dtypes.float8_e4m3))
    id8 = np.ascontiguousarray(np.eye(128, dtype=np.float32)
                               .astype(ml_dtypes.float8_e4m3))

    # sll_w1 + bias, padded to 2048 rows, as [k, (t d)]
    w1cat = np.zeros((KLAB * 128, D), np.float32)
    w1cat[0:LAB] = f(inputs["sll_w1"])
    w1cat[LAB] = f(inputs["sll_b1"])
    w1sbH = bf(w1cat.reshape(KLAB, 128, D).transpose(1, 0, 2)
               .reshape(128, KLAB * D))

    glu_w = f(inputs["glu_w"])
    gwg, gwt = glu_w[0:GLU], glu_w[GLU:2 * GLU]
    wbdH = np.zeros((128, 16 * H), np.float32)
    for jl in range(8):
        wbdH[jl * GLU:(jl + 1) * GLU, jl * H:(jl + 1) * H] = gwg
        wbdH[jl * GLU:(jl + 1) * GLU, (8 + jl) * H:(9 + jl) * H] = gwt
    gb8H = np.tile(f(inputs["glu_b"]).reshape(1, H), (1, 8))
    rowsB = np.zeros((1, 8 * H + H), np.float32)
    rowsB[0, 0:8 * H] = gb8H
    rowsB[0, 8 * H:] = f(inputs["glu_gate"])

    medw = f(inputs["med_w"])
    mw2 = cat([medw[128:MED], f(inputs["med_b"]).reshape(1, D)], 0)

    def headT(w):  # [64, 64] -> [c, (h d)] with w^T per head block
        wt = f(w).T.reshape(NH, DH, D)
        return np.ascontiguousarray(wt.transpose(1, 0, 2).reshape(DH, NH * D))

    packH = np.zeros((128, PCOLS), np.float32)

    def put(name, arr):
        lo, hi = _PC[name]
        arr = np.asarray(arr, np.float32)
        packH[0:arr.shape[0], lo:hi] = arr

    put("ident", np.eye(128, dtype=np.float32))
    put("woT", f(inputs["m1_wo"]).T)
    put("m2wvT", f(inputs["m2_wv"]).T)
    put("m2wo", f(inputs["m2_wo"]))
    put("mwsb", medw[0:128])
    put("mw2sb", mw2)
    put("mgT", f(inputs["med_gate"]).reshape(D, 1))
    put("w2sb", cat([f(inputs["sll_w2"]), f(inputs["sll_b2"]).reshape(1, H)], 0))
    put("gw3", cat([gwg, gwt], 1))
    put("wqT4", headT(inputs["m1_wq"]))
    put("wkT4", headT(inputs["m1_wk"]))
    put("wvT4", headT(inputs["m1_wv"]))
    put("wbd", wbdH)

    # out_w2 + bias, padded to 1280 rows, as [k, (t n)]
    w2cat = np.zeros((1280, MED), np.float32)
    w2cat[0:HID] = f(inputs["out_w2"])
    w2cat[HID] = f(inputs["out_b2"])
    ow2sbH = bf(w2cat.reshape(10, 128, MED).transpose(1, 0, 2)
                .reshape(128, 10 * MED))

    b1p = np.zeros(1280, np.float32)
    b1p[0:HID] = f(inputs["out_b1"])
    outb1T = np.ascontiguousarray(b1p.reshape(10, 128).T)

    rep = {
        "slab": slab, "slabV": slabV, "id8": id8, "packB": bf(packH), "rowsB": bf(rowsB),
        "w1sbH": w1sbH, "ow2sbH": ow2sbH, "outb1T": outb1T,
    }

    in_maps = []
    for c in range(NC_CORES):
        sl = slice(c * BC, (c + 1) * BC)
        labTc = np.zeros((KLAB * 128, BC), np.float32)
        labTc[0:LAB] = lab[sl].T
        labTc[LAB] = 1.0
        labTc = labTc.reshape(KLAB, 128, BC).transpose(1, 0, 2).reshape(128, KLAB * BC)
        gluTc = np.zeros((512, BC), np.float32)
        gluTc[0:T * GLU] = glu[sl].T
        gluTc = gluTc.reshape(4, 128, BC).transpose(1, 0, 2).reshape(128, 4 * BC)
        tfTc = np.zeros((512, BC), np.float32)
        tfTc[0:T * GLU] = tf[sl].T
        tfTc = tfTc.reshape(4, 128, BC).transpose(1, 0, 2).reshape(128, 4 * BC)
        med0Tc = np.ones((MED + 1, BC), np.float32)
        med0Tc[0:MED] = med0[sl].T
        in_maps.append({
            "labT": bf(labTc), "gluT": bf(gluTc), "tfT": bf(tfTc),
            "med0T": med0Tc, **rep,
        })
    return in_maps


def kernel(**inputs):
    global _CACHED_NC
    if _CACHED_NC is None:
        _CACHED_NC = build_bass()
    nc = _CACHED_NC
    in_maps = make_in_maps(inputs)
    res = run_bass_kernel_spmd(nc, in_maps, core_ids=list(range(NC_CORES)))
    return np.concatenate([res.results[c]["out"] for c in range(NC_CORES)],
                          axis=0)


if __name__ == "__main__":
    import reference
    inp = reference.setup_inputs()
    out = kernel(**{k: np.asarray(v) for k, v in inp.items()})
    print("kernel output", out.shape, out.dtype)


# revision 12
# speedup vs baseline: 1.1903x; 1.1903x over previous
"""Trainium2 Bass kernel for the MERITS_T patient model (B=1024 data-parallel
over 8 cores), collective-free.

Mathematical simplification of the reference (verified to ~3e-7 rel err fp32):
  - E_de softmaxes over a single key -> GATs / graph-MHA / drug_mem are dead
    code; e0 needs only attention query row 0, i.e. only med[:, 0, :].
  - The static half of patient_j is visit-independent -> softmax-invariant in
    the logits; its attention-weighted value is `static` and re-enters linearly
    via SS = sum_h MW_h[32:64].
  - The gate sigma(x.glu_gate) multiplies logits and values linearly; folded in
    as scalars around the softmax.
  - relu(final) @ out_w1 = relu(r) @ (sum_m out_w1[m]): the 43MB out_w1 only
    enters via its m-block sum W1sum [64, 1160].

Distribution: on this platform ANY firmware collective costs ~90us wall
(model-entry barrier + RDH protocol + launch skew; measured on a trivial
AllGather), and the remote-DMA ISA extension does not compile.  So the kernel
is fully data-parallel with zero cross-core traffic: every core reads the FULL
out_w1 in fp8 (x256, one e4m3 rounding; adds ~4.5e-3 output rel err vs the
2e-2 gate) and reduces the 145 m-blocks itself on the PE via identity-matmul
accumulation into PSUM (m-parity pairs folded into the 128 partitions; the
final MLP contracts both parities at once by duplicating relu(r) rows).
Everything else runs in bf16; sigmoids run as tanh on ScalarE (avoids an
activation-table swap before the Exp).
"""

import numpy as np
import ml_dtypes

import concourse.bass as bass
import concourse.mybir as mybir
from concourse.bass_utils import run_bass_kernel_spmd
from concourse.tile import TileContext

F32 = mybir.dt.float32
BF16 = mybir.dt.bfloat16
FP8 = mybir.dt.float8e4
AF = mybir.ActivationFunctionType
ALU = mybir.AluOpType
AX = mybir.AxisListType


def split_multi_waits(nc):
    """The walrus on this image encodes at most ONE sync wait per TPB
    instruction. Hoist excess waits onto standalone InstEventSemaphore ops."""
    wid = 0
    for f in nc.m.functions:
        for bb in f.blocks:
            out = []
            for ins in bb.instructions:
                si = ins.sync_info
                if si is not None and si.on_wait and len(si.on_wait) > 1:
                    waits = list(si.on_wait)
                    for w in waits[:-1]:
                        wid += 1
                        out.append(mybir.InstEventSemaphore(
                            name=f"Wsplit-{wid}", engine=ins.engine,
                            ins=[], outs=[],
                            sync_info=mybir.SyncInfo(on_wait=[w], on_update=[])))
                    si.on_wait = waits[-1:]
                out.append(ins)
            bb.instructions = out
    return wid


B, T, MED, LAB, GLU, D, H = 1024, 25, 145, 1956, 16, 64, 32
NH, DH = 4, 16
NC_CORES = 8
BC = B // NC_CORES       # 128 patients per core
HID = MED * D // 8       # 1160
KLAB = 16                # 2048 = 16*128 lab contraction tiles
MP = 73                  # m-pairs: 146 m-slots = 145 real + 1 zero pad
TP = T + 1               # 26, padded visit dim for the j-reduce
W1SCALE = 256.0          # fp8 pre-scale (out_w1 sigma~0.01 is subnormal in e4m3)

# column offsets inside the packed small-weight slab [128, PCOLS] (bf16)
_PC = {}
_o = 0
for _name, _w in [("ident", 128), ("woT", D), ("m2wvT", D), ("m2wo", D),
                  ("mwsb", D), ("mw2sb", D), ("mgT", 1), ("w2sb", H),
                  ("gw3", 2 * H), ("wqT4", NH * D), ("wkT4", NH * D),
                  ("wvT4", NH * D), ("wbd", 16 * H)]:
    _PC[_name] = (_o, _o + _w)
    _o += _w
PCOLS = _o


def build_bass():
    nc = bass.Bass()

    def inp(name, shape, dt=F32):
        return nc.dram_tensor(name, list(shape), dt, kind="ExternalInput")

    slab_d = inp("slab", (128, MP * 1024), FP8)     # out_w1 cols 0:1024, fp8 x256
    slabV_d = inp("slabV", (128, 136 * MP), FP8)   # cols 1024:1160, mp innermost
    id8_d = inp("id8", (128, 128), FP8)            # fp8 identity
    labT_d = inp("labT", (128, KLAB * BC), BF16)    # lab^T partition-major
    gluT_d = inp("gluT", (128, 4 * BC), BF16)       # glu partition-major
    tfT_d = inp("tfT", (128, 4 * BC), BF16)         # time_feat, same layout
    med0T_d = inp("med0T", (MED + 1, BC))           # med visit-0 ^T + ones row
    packB_d = inp("packB", (128, PCOLS), BF16)      # small weights, packed
    rows_d = inp("rowsB", (1, 8 * H + H), BF16)     # glu_b x8 | glu_gate
    outb1_d = inp("outb1T", (128, 10))              # out_b1 as [p, t]
    ow2sb_d = inp("ow2sbH", (128, 10 * MED), BF16)  # out_w2+b2 as [k, (t n)]
    w1sb_d = inp("w1sbH", (128, KLAB * D), BF16)    # sll_w1+b1 as [k, (t d)]
    out_d = nc.dram_tensor("out", [BC, MED], F32, kind="ExternalOutput")

    with TileContext(nc) as tc, \
            tc.tile_pool(name="consts", bufs=1) as cp, \
            tc.tile_pool(name="pa", bufs=3, space="PSUM") as pa, \
            tc.tile_pool(name="ps1", bufs=1, space="PSUM") as ps1, \
            tc.tile_pool(name="pw", bufs=1, space="PSUM") as pw:

        dmaA = nc.scalar.dma_start   # qAct ring: everything but the slab
        dmaS = nc.sync.dma_start     # qSP ring: the big fp8 slab + output

        # ================= input DMAs ====================================
        # all DRAM layouts are partition-major (host-marshalled), so every
        # transfer is a plain 2D copy with a short descriptor list.
        identF8 = cp.tile([128, 128], FP8, tag="identF8")
        dmaA(out=identF8, in_=id8_d[:])
        slab = cp.tile([128, MP, 1024], FP8, tag="slab")
        slab_v = slab_d[:].rearrange("p (m j) -> p m j", j=1024)
        slabV = cp.tile([128, 136, MP], FP8, tag="slabV")
        bnds = [0, 2, 5, 9, 14, 19, 24, 30, 36, 42, 48, 54, 60, 66, 73]
        NCH = len(bnds) - 1

        def slab_dma(q):
            lo, hi = bnds[q], bnds[q + 1]
            dmaS(out=slab[:, lo:hi, :], in_=slab_v[:, lo:hi, :])

        slab_dma(0)
        slab_dma(1)
        gluT = cp.tile([128, 4, BC], BF16, tag="gluT")
        dmaA(out=gluT, in_=gluT_d[:].rearrange("k (c p) -> k c p", p=BC))
        tfT = cp.tile([128, 4, BC], BF16, tag="tfT")
        dmaA(out=tfT, in_=tfT_d[:].rearrange("k (c p) -> k c p", p=BC))
        rows = cp.tile([1, 8 * H + H], BF16, tag="rows")
        dmaA(out=rows, in_=rows_d[:])
        ggb = cp.tile([128, H], BF16, tag="ggb")
        dmaA(out=ggb, in_=rows_d[0:1, 8 * H:8 * H + H].broadcast_to((128, H)))
        pack = cp.tile([128, PCOLS], BF16, tag="pack")
        dmaA(out=pack, in_=packB_d[:])
        med0Ta = cp.tile([128, BC], F32, tag="med0Ta")
        dmaA(out=med0Ta, in_=med0T_d[0:128, :])
        med0Tb = cp.tile([18, BC], F32, tag="med0Tb")
        dmaA(out=med0Tb, in_=med0T_d[128:MED + 1, :])
        slab_dma(2)
        w1sb = cp.tile([128, KLAB, D], BF16, tag="w1sb")
        dmaA(out=w1sb, in_=w1sb_d[:].rearrange("k (t d) -> k t d", d=D))
        labT = cp.tile([128, KLAB, BC], BF16, tag="labT")
        dmaA(out=labT, in_=labT_d[:].rearrange("k (t p) -> k t p", p=BC))
        outb1T = cp.tile([128, 10], F32, tag="outb1T")
        dmaA(out=outb1T, in_=outb1_d[:])
        slabV_v = slabV_d[:].rearrange("k (j m) -> k j m", m=MP)
        dmaA(out=slabV[:, 0:68, :], in_=slabV_v[:, 0:68, :])
        dmaA(out=slabV[:, 68:136, :], in_=slabV_v[:, 68:136, :])
        for q in range(3, NCH):
            slab_dma(q)
        ow2sb = cp.tile([128, 10, MED], BF16, tag="ow2sb")
        dmaA(out=ow2sb, in_=ow2sb_d[:].rearrange("k (t n) -> k t n", n=MED))

        def pk(name, nrows):
            lo, hi = _PC[name]
            return pack[0:nrows, lo:hi]

        identb = pk("ident", 128)
        wbd = pk("wbd", 128).rearrange("k (t h) -> k t h", h=H)
        gw3 = pk("gw3", GLU)
        gb8 = rows[0:1, 0:8 * H]

        ones1b = cp.tile([1, 128], BF16, tag="ones1b")
        nc.vector.memset(ones1b, 1.0)

        # ================= W1sum accumulate helper =======================
        acc = pw.tile([128, 1024], F32, tag="acc")
        JCH = [(0, 512), (512, 1024)]

        def slab_block(q):
            for mp in range(bnds[q], bnds[q + 1]):
                for (jl, jh) in JCH:
                    nc.tensor.matmul(acc[:, jl:jh], lhsT=identF8,
                                     rhs=slab[:, mp, jl:jh],
                                     start=(mp == 0), stop=(mp == MP - 1))

        slab_block(0)

        # ================= glu encoder x = tanh(glu_in @ glu_w + b) ======
        x_sbb = cp.tile([128, T, H], BF16, tag="x_sbb")
        xTb = cp.tile([128, H, TP], BF16, tag="xTb")
        nc.vector.memset(xTb[:, :, T:TP], 0.0)
        for c in range(3):
            gx = pa.tile([128, 8, H], F32, tag="pp")
            nc.tensor.matmul(gx, lhsT=gluT[:, c, :], rhs=wbd[:, 0:8, :],
                             start=True, stop=False)
            nc.tensor.matmul(gx, lhsT=tfT[:, c, :], rhs=wbd[:, 8:16, :],
                             start=False, stop=False)
            nc.tensor.matmul(gx, lhsT=ones1b[0:1, :],
                             rhs=gb8.rearrange("a (t h) -> a t h", h=H),
                             start=False, stop=True)
            nc.scalar.activation(out=x_sbb[:, 8 * c:8 * c + 8, :], in_=gx,
                                 func=AF.Tanh)
            nc.scalar.activation(out=xTb[:, :, 8 * c:8 * c + 8],
                                 in_=gx.rearrange("p j f -> p f j"),
                                 func=AF.Tanh)
        gx3 = pa.tile([128, 1, H], F32, tag="pp")
        nc.tensor.matmul(gx3[:, 0, :], lhsT=gluT[0:GLU, 3, :], rhs=gw3[:, 0:H],
                         start=True, stop=False)
        nc.tensor.matmul(gx3[:, 0, :], lhsT=tfT[0:GLU, 3, :], rhs=gw3[:, H:2 * H],
                         start=False, stop=False)
        nc.tensor.matmul(gx3[:, 0, :], lhsT=ones1b[0:1, :], rhs=gb8[0:1, 0:H],
                         start=False, stop=True)
        nc.scalar.activation(out=x_sbb[:, 24:25, :], in_=gx3, func=AF.Tanh)
        nc.scalar.activation(out=xTb[:, :, 24:25],
                             in_=gx3.rearrange("p a f -> p f a"), func=AF.Tanh)

        slab_block(1)

        # ================= weight prep on PE (bf16) ======================
        wvo_ps = pa.tile([D, D], F32, tag="pp")
        nc.tensor.matmul(wvo_ps, lhsT=pk("m2wvT", D), rhs=pk("m2wo", D))
        wvo2b = cp.tile([D, D], BF16, tag="wvo2b")
        nc.scalar.copy(out=wvo2b, in_=wvo_ps)
        woT = pk("woT", D)
        wov_ps = pa.tile([DH, NH, D], F32, tag="pp")
        for h in range(NH):
            nc.tensor.matmul(wov_ps[:, h, :], lhsT=woT[:, h * DH:(h + 1) * DH],
                             rhs=wvo2b[:])
        wov4 = cp.tile([DH, NH, D], BF16, tag="wov4")
        nc.scalar.copy(out=wov4, in_=wov_ps)
        wvT4 = pk("wvT4", DH).rearrange("c (h d) -> c h d", h=NH)
        mw_ps = pa.tile([H, NH, D], F32, tag="pp")
        for h in range(NH):
            nc.tensor.matmul(mw_ps[:, h, :], lhsT=wvT4[:, h, 0:H],
                             rhs=wov4[:, h, :])
        mw4b = cp.tile([H, NH, D], BF16, tag="mw4b")
        nc.scalar.copy(out=mw4b, in_=mw_ps)
        ss_ps = pa.tile([H, D], F32, tag="pp")
        for h in range(NH):
            nc.tensor.matmul(ss_ps, lhsT=wvT4[:, h, H:D], rhs=wov4[:, h, :],
                             start=(h == 0), stop=(h == NH - 1))
        ss_b = cp.tile([H, D], BF16, tag="ss_b")
        nc.scalar.copy(out=ss_b, in_=ss_ps)
        wqT4 = pk("wqT4", DH).rearrange("c (h d) -> c h d", h=NH)
        wkT4 = pk("wkT4", DH).rearrange("c (h d) -> c h d", h=NH)
        ahg_ps = pa.tile([D, NH, H], F32, tag="pp")
        for h in range(NH):
            nc.tensor.matmul(ahg_ps[:, h, :], lhsT=wqT4[:, h, :],
                             rhs=wkT4[:, h, 0:H])
        ahgb = cp.tile([D, NH, H], BF16, tag="ahgb")
        nc.scalar.activation(out=ahgb, in_=ahg_ps, func=AF.Copy,
                             scale=1.0 / DH ** 0.5)

        slab_block(2)

        # ================= med visit-0 encoder (transposed) ==============
        mbTa = cp.tile([128, BC], BF16, tag="mbTa")
        nc.vector.tensor_scalar(out=mbTa, in0=med0Ta, scalar1=0.9, scalar2=None,
                                op0=ALU.is_gt)
        mbTb = cp.tile([18, BC], BF16, tag="mbTb")
        nc.vector.tensor_scalar(out=mbTb, in0=med0Tb, scalar1=0.9, scalar2=None,
                                op0=ALU.is_gt)
        x0_ps = pa.tile([D, BC], F32, tag="pp")
        nc.tensor.matmul(x0_ps, lhsT=pk("mwsb", 128), rhs=mbTa[:],
                         start=True, stop=False)
        nc.tensor.matmul(x0_ps, lhsT=pk("mw2sb", 18), rhs=mbTb[:],
                         start=False, stop=True)
        x0b = cp.tile([D, BC], BF16, tag="x0b")
        nc.vector.tensor_copy(out=x0b, in_=x0_ps)
        g0_ps = pa.tile([1, BC], F32, tag="pp")
        nc.tensor.matmul(g0_ps, lhsT=pk("mgT", D), rhs=x0b[:])
        # sigmoid(z) = 0.5*tanh(z/2) + 0.5 (keeps ScalarE on the tanh table)
        tg0 = cp.tile([1, BC], F32, tag="tg0")
        nc.scalar.activation(out=tg0, in_=g0_ps, func=AF.Tanh, scale=0.5)
        sg0b = cp.tile([1, BC], BF16, tag="sg0b")
        nc.vector.tensor_scalar(out=sg0b, in0=tg0, scalar1=0.5, scalar2=0.5,
                                op0=ALU.mult, op1=ALU.add)
        sg0r_ps = pa.tile([D, BC], F32, tag="pp")
        nc.tensor.matmul(sg0r_ps, lhsT=ones1b[0:1, 0:D], rhs=sg0b[:])
        mr0b = cp.tile([D, BC], BF16, tag="mr0b")
        nc.vector.tensor_mul(mr0b, x0b, sg0r_ps)
        u_ps = pa.tile([BC, NH, H], F32, tag="pp")
        nc.tensor.matmul(u_ps, lhsT=mr0b[:],
                         rhs=ahgb[:].rearrange("d h f -> d (h f)"))
        u_bb = cp.tile([BC, NH, H], BF16, tag="u_bb")
        nc.vector.tensor_copy(out=u_bb, in_=u_ps)

        # ================= gate = sigmoid(x . glu_gate) ==================
        gm = cp.tile([128, T, H], BF16, tag="gm")
        nc.vector.tensor_mul(gm, x_sbb,
                             ggb[:].unsqueeze(1).broadcast_to((128, T, H)))
        gs = cp.tile([128, T], F32, tag="gs")
        nc.vector.tensor_reduce(out=gs, in_=gm, axis=AX.X, op=ALU.add)
        gth = cp.tile([128, T], F32, tag="gth")
        nc.scalar.activation(out=gth, in_=gs, func=AF.Tanh, scale=0.5)
        gate = cp.tile([128, T], F32, tag="gate")
        nc.vector.tensor_scalar(out=gate, in0=gth, scalar1=0.5, scalar2=0.5,
                                op0=ALU.mult, op1=ALU.add)

        # ================= one-query attention (glu half only) ===========
        sprod = cp.tile([128, NH, T, H], BF16, tag="sprod")
        nc.vector.tensor_mul(
            sprod,
            x_sbb[:].unsqueeze(1).broadcast_to((128, NH, T, H)),
            u_bb[:].unsqueeze(2).broadcast_to((128, NH, T, H)))
        s4 = cp.tile([128, NH, T], F32, tag="s4")
        nc.vector.tensor_reduce(out=s4.rearrange("p h j -> p (h j)"),
                                in_=sprod.rearrange("p h j f -> p (h j) f"),
                                axis=AX.X, op=ALU.add)
        sg4 = cp.tile([128, NH, T], F32, tag="sg4")
        nc.vector.tensor_mul(sg4, s4,
                             gate[:].unsqueeze(1).broadcast_to((128, NH, T)))
        es = cp.tile([128, NH, TP], BF16, tag="es")
        nc.vector.memset(es[:, :, T:TP], 0.0)
        nc.scalar.activation(out=es[:, :, 0:T], in_=sg4, func=AF.Exp)
        den = cp.tile([128, NH], F32, tag="den")
        nc.vector.tensor_reduce(out=den, in_=es[:, :, 0:T], axis=AX.X,
                                op=ALU.add)
        rden = cp.tile([128, NH], F32, tag="rden")
        nc.vector.reciprocal(out=rden, in_=den)
        cgb = cp.tile([128, NH, TP], BF16, tag="cgb")
        nc.vector.tensor_mul(cgb[:, :, 0:T], es[:, :, 0:T],
                             gate[:].unsqueeze(1).broadcast_to((128, NH, T)))
        coefb = cp.tile([128, NH, TP], BF16, tag="coefb")
        nc.vector.memset(coefb[:, :, T:TP], 0.0)
        nc.vector.tensor_mul(coefb[:, :, 0:T], cgb[:, :, 0:T],
                             rden[:].unsqueeze(2).broadcast_to((128, NH, T)))
        wprod = cp.tile([128, NH, H, TP], BF16, tag="wprod")
        nc.vector.tensor_mul(
            wprod,
            coefb[:].unsqueeze(2).broadcast_to((128, NH, H, TP)),
            xTb[:].unsqueeze(1).broadcast_to((128, NH, H, TP)))
        y4 = cp.tile([128, NH, H], F32, tag="y4")
        nc.vector.tensor_reduce(out=y4.rearrange("p h f -> p (h f)"),
                                in_=wprod.rearrange("p h f j -> p (h f) j"),
                                axis=AX.X, op=ALU.add)
        y4b = cp.tile([128, NH, H], BF16, tag="y4b")
        nc.vector.tensor_copy(out=y4b, in_=y4)
        accV = cp.tile([128, 136], F32, tag="accV")
        nc.vector.tensor_reduce(out=accV[:, 0:68], in_=slabV[:, 0:68, :],
                                axis=AX.X, op=ALU.add)
        nc.vector.tensor_reduce(out=accV[:, 68:136], in_=slabV[:, 68:136, :],
                                axis=AX.X, op=ALU.add)

        # ================= static MLP over lab ===========================
        st1_ps = ps1.tile([D, BC], F32, tag="st1")
        for t in range(KLAB):
            nc.tensor.matmul(st1_ps, lhsT=w1sb[:, t, :], rhs=labT[:, t, :],
                             start=(t == 0), stop=(t == KLAB - 1))
        st1rb = cp.tile([D + 1, BC], BF16, tag="st1rb")
        nc.scalar.activation(out=st1rb[0:D, :], in_=st1_ps, func=AF.Relu)
        nc.vector.memset(st1rb[D:D + 1, :], 1.0)
        st2_ps = pa.tile([H, BC], F32, tag="pp")
        nc.tensor.matmul(st2_ps, lhsT=pk("w2sb", D + 1), rhs=st1rb[:])
        staticb = cp.tile([H, BC], BF16, tag="staticb")
        nc.scalar.activation(out=staticb, in_=st2_ps, func=AF.Relu)

        for q in range(3, 10):
            slab_block(q)

        # attention tail interleaved while the last slab chunks stream in
        yt_ps = pa.tile([H, NH, BC], BF16, tag="pp")
        for h in range(NH):
            nc.tensor.transpose(yt_ps[:, h, :], y4b[:, h, :], identb[:])
        yT4b = cp.tile([H, NH, BC], BF16, tag="yT4b")
        nc.scalar.copy(out=yT4b, in_=yt_ps)
        rT_ps = pa.tile([D, BC], F32, tag="pp")
        for h in range(NH):
            nc.tensor.matmul(rT_ps, lhsT=mw4b[:, h, :], rhs=yT4b[:, h, :],
                             start=(h == 0), stop=False)
        nc.tensor.matmul(rT_ps, lhsT=ss_b[:], rhs=staticb[:],
                         start=False, stop=True)
        # relu(r)^T duplicated into both parity halves for the mm1 contraction
        rrT2 = cp.tile([128, BC], BF16, tag="rrT2")
        nc.scalar.activation(out=rrT2[0:D, :], in_=rT_ps, func=AF.Relu)
        nc.scalar.activation(out=rrT2[D:128, :], in_=rT_ps, func=AF.Relu)

        for q in range(10, NCH):
            slab_block(q)

        # ================= final MLP =====================================
        # PE-reduced cols (t=0..7) and DVE-reduced cols (t=8,9) live in
        # separate tiles so mm1 block deps stay decoupled.
        w1bbP = cp.tile([128, 1024], BF16, tag="w1bbP")
        w1bbV = cp.tile([128, 136], BF16, tag="w1bbV")
        nc.vector.tensor_copy(out=w1bbV, in_=accV)
        nc.scalar.copy(out=w1bbP[:, 0:512], in_=acc[:, 0:512])
        nc.scalar.copy(out=w1bbP[:, 512:1024], in_=acc[:, 512:1024])
        hidT = cp.tile([128, 10, BC], BF16, tag="hidT")
        nc.vector.memset(hidT[:, 9, :], 1.0)
        for t in range(10):
            n = 128 if t < 9 else 8
            h_ps = pa.tile([128, BC], F32, tag="pp")
            if t < 8:
                lhsT_t = w1bbP[:, t * 128:t * 128 + n]
            else:
                lhsT_t = w1bbV[:, (t - 8) * 128:(t - 8) * 128 + n]
            nc.tensor.matmul(h_ps[0:n, :], lhsT=lhsT_t, rhs=rrT2[:])
            if t % 2 == 0:
                nc.scalar.activation(out=hidT[0:n, t, :], in_=h_ps[0:n, :],
                                     func=AF.Relu, scale=1.0 / W1SCALE,
                                     bias=outb1T[0:n, t:t + 1])
            else:
                nc.vector.scalar_tensor_tensor(
                    out=hidT[0:n, t, :], in0=h_ps[0:n, :],
                    scalar=1.0 / W1SCALE,
                    in1=outb1T[0:n, t:t + 1].broadcast_to((n, BC)),
                    op0=ALU.mult, op1=ALU.add)
                nc.vector.tensor_scalar(out=hidT[0:n, t, :],
                                        in0=hidT[0:n, t, :], scalar1=0.0,
                                        scalar2=None, op0=ALU.max)
        out_ps = ps1.tile([BC, MED], F32, tag="st1")
        for t in range(10):
            k = 128 if t < 9 else 9
            nc.tensor.matmul(out_ps, lhsT=hidT[0:k, t, :], rhs=ow2sb[0:k, t, :],
                             start=(t == 0), stop=(t == 9))
        out_sb = cp.tile([BC, MED], F32, tag="out_sb")
        nc.vector.tensor_copy(out=out_sb, in_=out_ps)
        dmaS(out=out_d[:], in_=out_sb)

    split_multi_waits(nc)
    return nc


_CACHED_NC = None


def make_in_maps(inputs):
    """Pure input marshalling: transpose / reshape / concat / pad / cast only."""
    f = lambda x: np.ascontiguousarray(np.asarray(x, dtype=np.float32))
    cat = np.concatenate
    bf = lambda x: np.ascontiguousarray(np.asarray(x).astype(ml_dtypes.bfloat16))

    lab = f(inputs["lab"])
    glu = f(inputs["glu"]).reshape(B, T * GLU)
    tf = f(inputs["time_feat"]).reshape(B, T * GLU)
    med0 = f(inputs["med"])[:, 0, :]

    # full out_w1 -> fp8 x256, m-parity folded into partitions: [128, mp, j]
    w1b = f(inputs["out_w1"]).reshape(MED, D, HID)
    w1p = np.zeros((2 * MP, D, HID), np.float32)
    w1p[0:MED] = w1b
    arr = (w1p.reshape(MP, 2, D, HID).transpose(1, 2, 0, 3)
           .reshape(128, MP, HID) * W1SCALE)
    slab = np.ascontiguousarray(
        arr[:, :, 0:1024].reshape(128, MP * 1024)
        .astype(ml_dtypes.float8_e4m3))
    slabV = np.ascontiguousarray(
        arr[:, :, 1024:HID].transpose(0, 2, 1).reshape(128, 136 * MP)
        .astype(ml_dtypes.float8_e4m3))
    id8 = np.ascontiguousarray(np.eye(128, dtype=np.float32)
                               .astype(ml_dtypes.float8_e4m3))

    # sll_w1 + bias, padded to 2048 rows, as [k, (t d)]
    w1cat = np.zeros((KLAB * 128, D), np.float32)
    w1cat[0:LAB] = f(inputs["sll_w1"])
    w1cat[LAB] = f(inputs["sll_b1"])
    w1sbH = bf(w1cat.reshape(KLAB, 128, D).transpose(1, 0, 2)
               .reshape(128, KLAB * D))

    glu_w = f(inputs["glu_w"])
    gwg, gwt = glu_w[0:GLU], glu_w[GLU:2 * GLU]
    wbdH = np.zeros((128, 16 * H), np.float32)
    for jl in range(8):
        wbdH[jl * GLU:(jl + 1) * GLU, jl * H:(jl + 1) * H] = gwg
        wbdH[jl * GLU:(jl + 1) * GLU, (8 + jl) * H:(9 + jl) * H] = gwt
    gb8H = np.tile(f(inputs["glu_b"]).reshape(1, H), (1, 8))
    rowsB = np.zeros((1, 8 * H + H), np.float32)
    rowsB[0, 0:8 * H] = gb8H
    rowsB[0, 8 * H:] = f(inputs["glu_gate"])

    medw = f(inputs["med_w"])
    mw2 = cat([medw[128:MED], f(inputs["med_b"]).reshape(1, D)], 0)

    def headT(w):  # [64, 64] -> [c, (h d)] with w^T per head block
        wt = f(w).T.reshape(NH, DH, D)
        return np.ascontiguousarray(wt.transpose(1, 0, 2).reshape(DH, NH * D))

    packH = np.zeros((128, PCOLS), np.float32)

    def put(name, arr):
        lo, hi = _PC[name]
        arr = np.asarray(arr, np.float32)
        packH[0:arr.shape[0], lo:hi] = arr

    put("ident", np.eye(128, dtype=np.float32))
    put("woT", f(inputs["m1_wo"]).T)
    put("m2wvT", f(inputs["m2_wv"]).T)
    put("m2wo", f(inputs["m2_wo"]))
    put("mwsb", medw[0:128])
    put("mw2sb", mw2)
    put("mgT", f(inputs["med_gate"]).reshape(D, 1))
    put("w2sb", cat([f(inputs["sll_w2"]), f(inputs["sll_b2"]).reshape(1, H)], 0))
    put("gw3", cat([gwg, gwt], 1))
    put("wqT4", headT(inputs["m1_wq"]))
    put("wkT4", headT(inputs["m1_wk"]))
    put("wvT4", headT(inputs["m1_wv"]))
    put("wbd", wbdH)

    # out_w2 + bias, padded to 1280 rows, as [k, (t n)]
    w2cat = np.zeros((1280, MED), np.float32)
    w2cat[0:HID] = f(inputs["out_w2"])
    w2cat[HID] = f(inputs["out_b2"])
    ow2sbH = bf(w2cat.reshape(10, 128, MED).transpose(1, 0, 2)
                .reshape(128, 10 * MED))

    b1p = np.zeros(1280, np.float32)
    b1p[0:HID] = f(inputs["out_b1"])
    outb1T = np.ascontiguousarray(b1p.reshape(10, 128).T)

    rep = {
        "slab": slab, "slabV": slabV, "id8": id8, "packB": bf(packH), "rowsB": bf(rowsB),
        "w1sbH": w1sbH, "ow2sbH": ow2sbH, "outb1T": outb1T,
    }

    in_maps = []
    for c in range(NC_CORES):
        sl = slice(c * BC, (c + 1) * BC)
        labTc = np.zeros((KLAB * 128, BC), np.float32)
        labTc[0:LAB] = lab[sl].T
        labTc[LAB] = 1.0
        labTc = labTc.reshape(KLAB, 128, BC).transpose(1, 0, 2).reshape(128, KLAB * BC)
        gluTc = np.zeros((512, BC), np.float32)
        gluTc[0:T * GLU] = glu[sl].T
        gluTc = gluTc.reshape(4, 128, BC).transpose(1, 0, 2).reshape(128, 4 * BC)
        tfTc = np.zeros((512, BC), np.float32)
        tfTc[0:T * GLU] = tf[sl].T
        tfTc = tfTc.reshape(4, 128, BC).transpose(1, 0, 2).reshape(128, 4 * BC)
        med0Tc = np.ones((MED + 1, BC), np.float32)
        med0Tc[0:MED] = med0[sl].T
        in_maps.append({
            "labT": bf(labTc), "gluT": bf(gluTc), "tfT": bf(tfTc),
            "med0T": med0Tc, **rep,
        })
    return in_maps


def kernel(**inputs):
    global _CACHED_NC
    if _CACHED_NC is None:
        _CACHED_NC = build_bass()
    nc = _CACHED_NC
    in_maps = make_in_maps(inputs)
    res = run_bass_kernel_spmd(nc, in_maps, core_ids=list(range(NC_CORES)))
    return np.concatenate([res.results[c]["out"] for c in range(NC_CORES)],
                          axis=0)


if __name__ == "__main__":
    import reference
    inp = reference.setup_inputs()
    out = kernel(**{k: np.asarray(v) for k, v in inp.items()})
    print("kernel output", out.shape, out.dtype)
